# revision 1
# baseline (speedup 1.0000x reference)
"""Trainium2 Bass kernel for nn_CustomCLIP_11407433138213 (moe_routing).

Math (per sample b with domain n = labels[b]):
    h   = relu(x @ W1[n])                 [R]
    a   = relu(h @ W2[n])                 [D]
    f   = 0.2*a + 0.8*x                   [D]
    out = exp(ls) * (f/||f||) @ T^T       [N_TXT]

Device strategy (data-parallel over batch, 8 cores x 2048 rows):
  Everything is computed transposed (samples on the free dim):
    - XT [D, rows] streamed per row-block of 512.
    - mm1: hT[n] = W1[n]^T XT  (PSUM, K-chunks of 128), per expert.
    - g[n] = relu(hT[n]) * bcast(mask[n])   (one DVE scalar_tensor_tensor)
    - mm2: pa[d] = sum_n W2'[n]^T g[n]  with W2' = 0.25*W2 (all experts
      accumulate into one PSUM tile; one-hot masks make it a select).
    - f'[d] = relu(pa[d]) + XT[d]  == (0.2*a + 0.8*x)/0.8 per column.
    - s = colsum(f'^2) via ones-matmul; inv = 1/sqrt(s*exp(-2 ls));
      bcast(inv) via rank-1 matmul; fs = f' * bcast(inv).
    - mm3: logitsT[t] = TT^T fs  -> DRAM [N_TXT_PAD, rows].
  Host: transpose/shard inputs, one-hot masks, 0.25*W2, pad text to 1408,
  then gather logits[rows, txt] from per-core logitsT.
Matmuls run as float32r (full PE rate at N=512; fp32 would be 4x slower).
Emission is software-pipelined: block b+1's mm1/mm2 are emitted between
block b's norm reduction and its use, so the PE never waits on the
ACT/DVE norm chain.
"""

import contextlib
import os
import sys

sys.path.insert(0, "/opt/trn_rl_repo")

import numpy as np

import concourse.bass as bass  # noqa: F401  (registers engine types)
import concourse.mybir as mybir
import concourse.tile as tile
from concourse import bacc
from concourse.bass_utils import run_bass_kernel_spmd

# Problem constants (hardcoded per task contract).
B, D, R, ND, NT = 16384, 1024, 256, 3, 1380
NC = 8                    # cores
BPC = B // NC             # rows per core = 2048
RB = 512                  # row-block (matmul moving dim)
NB = BPC // RB            # row-blocks per core = 4
KD = D // 128             # 8 contraction chunks over D
KR = R // 128             # 2 chunks over R
MR = R // 128             # 2 M-chunks over R
NTP = 1408                # text padded to 11*128
TTI = NTP // 128          # 11 text chunks

F32 = mybir.dt.float32
MM_DT = mybir.dt.float32 if os.environ.get("KMM_DT") == "f32" else mybir.dt.float32r


def _f32(ap):
    """View a matmul-dtype AP as plain fp32 for ACT/DVE consumption."""
    return ap.bitcast(F32) if MM_DT != F32 else ap


def build_program():
    nc = bacc.Bacc(
        "TRN2",
        target_bir_lowering=False,
        debug=False,
        enable_asserts=True,
        num_devices=NC,
    )
    xt = nc.declare_dram_parameter("xt", [D, BPC], MM_DT, isOutput=False)
    msk = nc.declare_dram_parameter("msk", [ND, BPC], F32, isOutput=False)
    w1 = nc.declare_dram_parameter("w1", [ND, D, R], MM_DT, isOutput=False)
    w2 = nc.declare_dram_parameter("w2", [ND, R, D], MM_DT, isOutput=False)
    tt = nc.declare_dram_parameter("tt", [D, NTP], MM_DT, isOutput=False)
    sc = nc.declare_dram_parameter("sc", [1, 1], F32, isOutput=False)
    oc = nc.declare_dram_parameter("oc", [128, 1], MM_DT, isOutput=False)
    orow = nc.declare_dram_parameter("orow", [1, 128], MM_DT, isOutput=False)
    ot = nc.declare_dram_parameter("ot", [NTP, BPC], F32, isOutput=True)

    with tile.TileContext(nc) as tc, contextlib.ExitStack() as ctx:
        cst = ctx.enter_context(tc.tile_pool(name="cst", bufs=1))
        p_xb = ctx.enter_context(tc.tile_pool(name="p_xb", bufs=16))
        p_mb = ctx.enter_context(tc.tile_pool(name="p_mb", bufs=5))
        p_g = ctx.enter_context(tc.tile_pool(name="p_g", bufs=6))
        p_fp = ctx.enter_context(tc.tile_pool(name="p_fp", bufs=16))
        p_sq = ctx.enter_context(tc.tile_pool(name="p_sq", bufs=3))
        p_acc = ctx.enter_context(tc.tile_pool(name="p_acc", bufs=2))
        p_pbs = ctx.enter_context(tc.tile_pool(name="p_pbs", bufs=2))
        p_ob = ctx.enter_context(tc.tile_pool(name="p_ob", bufs=2))
        p_nrm = ctx.enter_context(tc.tile_pool(name="p_nrm", bufs=1))

        ps_h = ctx.enter_context(tc.tile_pool(name="ps_h", bufs=2, space="PSUM"))
        ps_a = ctx.enter_context(tc.tile_pool(name="ps_a", bufs=2, space="PSUM"))
        ps_s = ctx.enter_context(tc.tile_pool(name="ps_s", bufs=1, space="PSUM"))
        ps_l = ctx.enter_context(tc.tile_pool(name="ps_l", bufs=2, space="PSUM"))

        # ---- constant tiles (loads emitted in stages below) -------------
        w1t = [
            [
                cst.tile([128, R], MM_DT, name=f"w1_{n}_{k}", tag=f"w1_{n}_{k}")
                for k in range(KD)
            ]
            for n in range(ND)
        ]
        w2t = [
            [
                cst.tile([128, D], MM_DT, name=f"w2_{n}_{r}", tag=f"w2_{n}_{r}")
                for r in range(KR)
            ]
            for n in range(ND)
        ]
        ttt = [
            cst.tile([128, NTP], MM_DT, name=f"tt_{k}", tag=f"tt_{k}")
            for k in range(KD)
        ]
        ones_col = cst.tile([128, 1], MM_DT, name="ones_col", tag="ones_col")
        ones_row = cst.tile([1, 128], MM_DT, name="ones_row", tag="ones_row")
        sct = cst.tile([1, 1], F32, name="sct", tag="sct")

        # per-block live tiles
        S = [dict() for _ in range(NB)]

        def emit_w1_loads():
            nc.sync.dma_start(ones_col[:], oc[:])
            nc.sync.dma_start(ones_row[:], orow[:])
            nc.sync.dma_start(sct[:], sc[:])

        def emit_w1_n(n):
            for k in range(KD):
                nc.sync.dma_start(w1t[n][k][:], w1[n, k * 128 : (k + 1) * 128, :])

        def emit_w2_loads():
            for n in range(ND):
                for r in range(KR):
                    nc.sync.dma_start(w2t[n][r][:], w2[n, r * 128 : (r + 1) * 128, :])

        def emit_tt_loads():
            for k in range(KD):
                nc.sync.dma_start(ttt[k][:], tt[k * 128 : (k + 1) * 128, :])

        def emit_loads(b):
            c0 = b * RB
            xb = []
            for k in range(KD):
                t = p_xb.tile([128, RB], MM_DT, name="xb", tag="xb")
                nc.sync.dma_start(t[:], xt[k * 128 : (k + 1) * 128, c0 : c0 + RB])
                xb.append(t)
            mb = []
            for n in range(ND):
                t = p_mb.tile([128, RB], F32, name="mb", tag="mb")
                nc.sync.dma_start(
                    t[:],
                    msk[n, c0 : c0 + RB]
                    .rearrange("(a n) -> a n", a=1)
                    .to_broadcast((128, RB)),
                )
                mb.append(t)
            S[b]["xb"] = xb
            S[b]["mb"] = mb

        def emit_mm1_g(b):
            xb, mb = S[b]["xb"], S[b]["mb"]
            g = [[None] * MR for _ in range(ND)]
            for n in range(ND):
                for m in range(MR):
                    ph = ps_h.tile([128, RB], F32, name="ph", tag="ph")
                    for k in range(KD):
                        nc.tensor.matmul(
                            ph[:],
                            w1t[n][k][:, m * 128 : (m + 1) * 128],
                            xb[k][:],
                            start=(k == 0),
                            stop=(k == KD - 1),
                        )
                    gt = p_g.tile([128, RB], MM_DT, name="g", tag="g")
                    nc.vector.scalar_tensor_tensor(
                        gt[:],
                        ph[:],
                        0.0,
                        mb[n][:],
                        mybir.AluOpType.max,
                        mybir.AluOpType.mult,
                    )
                    g[n][m] = gt
            S[b]["g"] = g

        def emit_mm2(b):
            xb, g = S[b]["xb"], S[b]["g"]
            fp = []
            sq = []
            for d in range(KD):
                pa = ps_a.tile([128, RB], F32, name="pa", tag="pa")
                first = True
                for n in range(ND):
                    for r in range(KR):
                        nc.tensor.matmul(
                            pa[:],
                            w2t[n][r][:, d * 128 : (d + 1) * 128],
                            g[n][r][:],
                            start=first,
                            stop=(n == ND - 1 and r == KR - 1),
                        )
                        first = False
                ft = p_fp.tile([128, RB], MM_DT, name="fp", tag="fp")
                nc.vector.scalar_tensor_tensor(
                    ft[:],
                    pa[:],
                    0.0,
                    _f32(xb[d][:]),
                    mybir.AluOpType.max,
                    mybir.AluOpType.add,
                )
                fp.append(ft)
                st = p_sq.tile([128, RB], F32, name="sq", tag="sq")
                nc.scalar.square(st[:], _f32(ft[:]))
                if d == 0:
                    acc = p_acc.tile([128, RB], F32, name="acc", tag="acc")
                    nc.gpsimd.tensor_copy(acc[:], st[:])
                elif d < KD - 1:
                    nc.gpsimd.tensor_add(acc[:], acc[:], st[:])
                else:
                    accm = p_acc.tile([128, RB], MM_DT, name="accm", tag="accm")
                    nc.gpsimd.tensor_add(accm[:], acc[:], st[:])
            S[b]["fp"] = fp
            S[b]["accm"] = accm

        def emit_ps_norm(b):
            accm = S[b]["accm"]
            ps = ps_s.tile([1, RB], F32, name="ps", tag="ps")
            nc.tensor.matmul(ps[:], ones_col[:], accm[:], start=True, stop=True)
            iv = p_nrm.tile([1, RB], MM_DT, name="iv", tag="iv")
            nc.scalar.activation(
                iv[:],
                ps[:],
                mybir.ActivationFunctionType.Abs_reciprocal_sqrt,
                scale=sct[:],
            )
            S[b]["iv"] = iv

        def emit_pb(b):
            iv = S[b]["iv"]
            pb = ps_l.tile([128, RB], F32, name="pl", tag="pl")
            nc.tensor.matmul(pb[:], ones_row[:], iv[:], start=True, stop=True)
            pbs = p_pbs.tile([128, RB], F32, name="pbs", tag="pbs")
            nc.scalar.copy(pbs[:], pb[:])
            S[b]["pbs"] = pbs

        def emit_mm3(b):
            c0 = b * RB
            fp = S[b]["fp"]
            pbs = S[b]["pbs"]
            for t_i in range(TTI):
                pl = ps_l.tile([128, RB], F32, name="pl", tag="pl")
                for k in range(KD):
                    nc.tensor.matmul(
                        pl[:],
                        ttt[k][:, t_i * 128 : (t_i + 1) * 128],
                        fp[k][:],
                        start=(k == 0),
                        stop=(k == KD - 1),
                    )
                ob = p_ob.tile([128, RB], F32, name="ob", tag="ob")
                nc.vector.tensor_mul(ob[:], pl[:], pbs[:])
                nc.sync.dma_start(
                    ot[t_i * 128 : (t_i + 1) * 128, c0 : c0 + RB], ob[:]
                )
            # drop per-block state (frees python refs only)
            S[b].clear()

        # ---- emission schedule (software pipelined) ---------------------
        # First wave: interleave W1[0] with xb(0) so the PE can start as
        # soon as the first weight + x chunks land.
        emit_w1_loads()
        emit_w1_n(0)
        emit_loads(0)
        emit_w1_n(1)
        emit_w1_n(2)
        emit_w2_loads()
        emit_mm1_g(0)
        emit_tt_loads()
        emit_mm2(0)
        emit_ps_norm(0)
        for b in range(NB):
            if b + 1 < NB:
                emit_loads(b + 1)
                emit_mm1_g(b + 1)
            emit_pb(b)
            if b + 1 < NB:
                emit_mm2(b + 1)
            emit_mm3(b)
            if b + 1 < NB:
                emit_ps_norm(b + 1)

    nc.compile()
    return nc


_NC_CACHE = None


def _get_program():
    global _NC_CACHE
    if _NC_CACHE is None:
        _NC_CACHE = build_program()
    return _NC_CACHE


def make_in_maps(image_features, domain_labels, W1, W2, text_features, logit_scale):
    image_features = np.asarray(image_features, dtype=np.float32)
    labels = np.asarray(domain_labels)
    W1 = np.asarray(W1, dtype=np.float32)
    W2 = np.asarray(W2, dtype=np.float32)
    text_features = np.asarray(text_features, dtype=np.float32)
    ls = float(np.asarray(logit_scale))

    # Host-side shard prep.
    xt_full = np.ascontiguousarray(image_features.T)            # [D, B]
    masks = np.zeros((ND, B), dtype=np.float32)
    masks[labels.astype(np.int64), np.arange(B)] = 1.0
    w2s = (0.25 * W2).astype(np.float32)                        # fold 0.2/0.8
    tt_full = np.zeros((D, NTP), dtype=np.float32)
    tt_full[:, :NT] = text_features.T
    sc = np.array([[np.exp(-2.0 * ls)]], dtype=np.float32)
    oc = np.ones((128, 1), dtype=np.float32)
    orow = np.ones((1, 128), dtype=np.float32)

    in_maps = []
    for c in range(NC):
        cols = slice(c * BPC, (c + 1) * BPC)
        in_maps.append(
            {
                "xt": np.ascontiguousarray(xt_full[:, cols]),
                "msk": np.ascontiguousarray(masks[:, cols]),
                "w1": W1,
                "w2": w2s,
                "tt": tt_full,
                "sc": sc,
                "oc": oc,
                "orow": orow,
            }
        )
    return in_maps


def kernel(image_features, domain_labels, W1, W2, text_features, logit_scale, **kw):
    in_maps = make_in_maps(
        image_features, domain_labels, W1, W2, text_features, logit_scale
    )
    nc = _get_program()
    res = run_bass_kernel_spmd(nc, in_maps, list(range(NC)))

    out = np.empty((B, NT), dtype=np.float32)
    for c in range(NC):
        out[c * BPC : (c + 1) * BPC, :] = res.results[c]["ot"][:NT, :].T
    return out



# revision 22
# speedup vs baseline: 1.3026x; 1.3026x over previous
"""Trainium2 Bass kernel for nn_CustomCLIP_11407433138213 (moe_routing).

Math (per sample b with domain n = labels[b]):
    h   = relu(x @ W1[n])                 [R]
    a   = relu(h @ W2[n])                 [D]
    f   = 0.2*a + 0.8*x                   [D]
    out = exp(ls) * (f/||f||) @ T^T       [N_TXT]

Device strategy (expert-sorted data parallel, 8 cores x 2048 rows):
  The host sorts rows by domain and packs each core's 2048 rows as
  3 "pure" blocks of 512 rows (single expert each) plus 1 "tail"
  block holding at most 2 domains (selected via 0/1 masks). Expert
  weights are gathered per block on the host, so routing is pure
  data and a single SPMD program serves all cores. This runs mm1/mm2
  once per row (the baseline ran all 3 experts for every row).

  Everything is computed transposed (samples on the free dim):
    - XT [D, rows] per row-block of 512, bf16.
    - pure block: hT = W1_blk^T XT (PSUM, K-chunks of 128);
      g = relu(hT); pa = W2_blk'^T g with W2' = 0.25*W2.
    - tail block: two experts, g_e = relu(hT_e) * bcast(mask_e),
      both accumulate into one PSUM tile (masks make it a select).
    - f'[d] = relu(pa[d]) + XT[d]  == (0.2*a + 0.8*x)/0.8 per column.
    - sq_d = f'^2 (ACT); ps += ones^T sq_d accumulated in PSUM by the
      PE per d-chunk (no cross-engine add chain);
      iv = 1/sqrt(ps*exp(-2 ls)); bcast(iv) via rank-1 matmul;
      logitsT = (TT^T f') * bcast(iv) -> DRAM [N_TXT_PAD, rows].
  Host: sort/pack rows, gather+scale weights, pad text to 1408,
  bf16-cast matmul operands, then unscatter logits[rows, txt].
Matmuls run in bf16 (1 PE cycle/row; rel-err budget 2e-2 allows it).
Emission is software-pipelined as in the baseline: block b+1's
mm1/mm2 are emitted between block b's norm reduction and its use.
"""

import contextlib
import sys

sys.path.insert(0, "/opt/trn_rl_repo")

import ml_dtypes
import numpy as np

import concourse.bass as bass  # noqa: F401  (registers engine types)
import concourse.mybir as mybir
import concourse.tile as tile
from concourse import bacc
from concourse.bass_utils import run_bass_kernel_spmd

# Problem constants (hardcoded per task contract).
B, D, R, ND, NT = 16384, 1024, 256, 3, 1380
NC = 8                    # cores
BPC = B // NC             # rows per core = 2048
RB = 512                  # row-block (matmul moving dim)
NB = BPC // RB            # row-blocks per core = 4
NPURE = NB - 1            # pure blocks per core = 3
KD = D // 128             # 8 contraction chunks over D
KR = R // 128             # 2 chunks over R
MR = R // 128             # 2 M-chunks over R
NTP = 1408                # text padded to 11*128
TTI = NTP // 128          # 11 text chunks

F32 = mybir.dt.float32
F32R = mybir.dt.float32r
BF16 = mybir.dt.bfloat16
MM_DT = BF16
NP_BF16 = ml_dtypes.bfloat16


def _f32(ap):
    """View an f32r AP as plain fp32 for ACT/DVE consumption."""
    return ap.bitcast(F32) if ap.dtype == F32R else ap


def build_program(sc_val, dbg=False):
    nc = bacc.Bacc(
        "TRN2",
        target_bir_lowering=False,
        debug=False,
        enable_asserts=True,
        num_devices=NC,
    )
    dbg_params = {}
    if dbg:
        for name, shape, dt in [
            ("dbg_xb0", [128, RB], F32),
            ("dbg_w100", [128, R], F32),
            ("dbg_tt0", [128, NTP], F32),
            ("dbg_ph0", [128, RB], F32),
            ("dbg_g00", [128, RB], F32),
            ("dbg_fp00", [128, RB], F32),
            ("dbg_sq00", [128, RB], F32),
            ("dbg_iv0", [1, RB], F32),
            ("dbg_pbs0", [128, RB], F32),
        ]:
            dbg_params[name] = nc.declare_dram_parameter(
                name, shape, dt, isOutput=True
            )
    xt = nc.declare_dram_parameter("xt", [D, BPC], BF16, isOutput=False)
    w1p = nc.declare_dram_parameter("w1p", [NB, D, R], BF16, isOutput=False)
    w1s = nc.declare_dram_parameter("w1s", [D, R], BF16, isOutput=False)
    w2p = nc.declare_dram_parameter("w2p", [NB, R, D], BF16, isOutput=False)
    w2s = nc.declare_dram_parameter("w2s", [R, D], BF16, isOutput=False)
    mk = nc.declare_dram_parameter("mk", [2, RB], F32, isOutput=False)
    tt = nc.declare_dram_parameter("tt", [D, NTP], BF16, isOutput=False)
    ot = nc.declare_dram_parameter("ot", [NTP, BPC], F32, isOutput=True)

    with tile.TileContext(nc) as tc, contextlib.ExitStack() as ctx:
        cst = ctx.enter_context(tc.tile_pool(name="cst", bufs=1))
        p_xb = ctx.enter_context(tc.tile_pool(name="p_xb", bufs=16))
        p_g = ctx.enter_context(tc.tile_pool(name="p_g", bufs=6))
        p_fp = ctx.enter_context(tc.tile_pool(name="p_fp", bufs=16))
        p_sq = ctx.enter_context(tc.tile_pool(name="p_sq", bufs=10))
        p_pbs = ctx.enter_context(tc.tile_pool(name="p_pbs", bufs=2))
        p_ob = ctx.enter_context(tc.tile_pool(name="p_ob", bufs=2))
        p_nrm = ctx.enter_context(tc.tile_pool(name="p_nrm", bufs=2))
        if dbg:
            p_dbg = ctx.enter_context(tc.tile_pool(name="p_dbg", bufs=1))

        def dbg_dump(name, ap):
            t = p_dbg.tile(list(ap.shape), F32, name=f"d_{name}", tag=f"d_{name}")
            nc.scalar.copy(t[:], ap)
            nc.sync.dma_start(dbg_params[name][:], t[:])

        ps_h = ctx.enter_context(tc.tile_pool(name="ps_h", bufs=2, space="PSUM"))
        ps_a = ctx.enter_context(tc.tile_pool(name="ps_a", bufs=2, space="PSUM"))
        ps_s = ctx.enter_context(tc.tile_pool(name="ps_s", bufs=1, space="PSUM"))
        ps_l = ctx.enter_context(tc.tile_pool(name="ps_l", bufs=2, space="PSUM"))

        # ---- constant tiles -------------------------------------------
        # per-block primary experts (idx 0..2 pure, 3 = tail expert A),
        # idx 4 = tail expert B.
        w1t = [
            [
                cst.tile([128, R], BF16, name=f"w1_{b}_{k}", tag=f"w1_{b}_{k}")
                for k in range(KD)
            ]
            for b in range(NB + 1)
        ]
        w2t = [
            [
                cst.tile([128, D], BF16, name=f"w2_{b}_{r}", tag=f"w2_{b}_{r}")
                for r in range(KR)
            ]
            for b in range(NB + 1)
        ]
        ttt = [
            cst.tile([128, NTP], BF16, name=f"tt_{k}", tag=f"tt_{k}")
            for k in range(KD)
        ]
        mkt = [
            cst.tile([128, RB], F32, name=f"mk_{e}", tag=f"mk_{e}") for e in range(2)
        ]
        ones_col = cst.tile([128, 1], BF16, name="ones_col", tag="ones_col")
        ones_row = cst.tile([1, 128], F32R, name="ones_row", tag="ones_row")
        ones_row32 = cst.tile([1, 128], F32, name="ones_row32", tag="ones_row32")

        # per-block live tiles
        S = [dict() for _ in range(NB)]

        def emit_const_loads():
            # memset, not DMA: tiny descriptors (a few bytes per partition)
            # corrupt subsequent bf16 DMA transfers on HW.
            nc.vector.memset(ones_col[:], 1.0)
            nc.vector.memset(ones_row32[:], 1.0)
            # DVE memset can't write f32r; ACT copy rounds to f32r.
            nc.scalar.copy(ones_row[:], ones_row32[:])
            for e in range(2):
                nc.sync.dma_start(
                    mkt[e][:],
                    mk[e, :].rearrange("(a n) -> a n", a=1).to_broadcast((128, RB)),
                )

        def emit_w_loads(b):
            """Weight loads for block-slot b (b == NB means tail expert B)."""
            for k in range(KD):
                src = (
                    w1s[k * 128 : (k + 1) * 128, :]
                    if b == NB
                    else w1p[b, k * 128 : (k + 1) * 128, :]
                )
                nc.sync.dma_start(w1t[b][k][:], src)
            for r in range(KR):
                src = (
                    w2s[r * 128 : (r + 1) * 128, :]
                    if b == NB
                    else w2p[b, r * 128 : (r + 1) * 128, :]
                )
                nc.sync.dma_start(w2t[b][r][:], src)

        def emit_tt_loads():
            for k in range(KD):
                nc.sync.dma_start(ttt[k][:], tt[k * 128 : (k + 1) * 128, :])

        def emit_loads(b):
            c0 = b * RB
            xb = []
            for k in range(KD):
                t = p_xb.tile([128, RB], BF16, name="xb", tag="xb")
                nc.sync.dma_start(t[:], xt[k * 128 : (k + 1) * 128, c0 : c0 + RB])
                xb.append(t)
            S[b]["xb"] = xb
            if dbg and b == 0:
                dbg_dump("dbg_xb0", xb[0][:])

        def emit_mm1_g(b):
            xb = S[b]["xb"]
            # expert slots contributing to this block: pure -> [b],
            # tail -> [3 (A), 4 (B)] with masks.
            slots = [b] if b < NPURE else [NB - 1, NB]
            g = []
            for si, s in enumerate(slots):
                for m in range(MR):
                    ph = ps_h.tile([128, RB], F32, name="ph", tag="ph")
                    for k in range(KD):
                        nc.tensor.matmul(
                            ph[:],
                            w1t[s][k][:, m * 128 : (m + 1) * 128],
                            xb[k][:],
                            start=(k == 0),
                            stop=(k == KD - 1),
                        )
                    if dbg and b == 0 and si == 0 and m == 0:
                        dbg_dump("dbg_ph0", ph[:])
                    gt = p_g.tile([128, RB], BF16, name="g", tag="g")
                    if b < NPURE:
                        nc.vector.tensor_scalar_max(gt[:], ph[:], 0.0)
                    else:
                        nc.vector.scalar_tensor_tensor(
                            gt[:],
                            ph[:],
                            0.0,
                            mkt[si][:],
                            mybir.AluOpType.max,
                            mybir.AluOpType.mult,
                        )
                    g.append((s, gt))
            S[b]["g"] = g
            if dbg and b == 0:
                dbg_dump("dbg_g00", g[0][1][:])

        def emit_mm2(b):
            xb, g = S[b]["xb"], S[b]["g"]
            fp = []
            sq = []
            ps = ps_s.tile([1, RB], F32, name="ps", tag="ps")
            for d in range(KD):
                pa = ps_a.tile([128, RB], F32, name="pa", tag="pa")
                ngrp = len(g)
                for gi, (s, gt) in enumerate(g):
                    r = gi % MR
                    nc.tensor.matmul(
                        pa[:],
                        w2t[s][r][:, d * 128 : (d + 1) * 128],
                        gt[:],
                        start=(gi == 0),
                        stop=(gi == ngrp - 1),
                    )
                ft = p_fp.tile([128, RB], BF16, name="fp", tag="fp")
                nc.vector.scalar_tensor_tensor(
                    ft[:],
                    pa[:],
                    0.0,
                    xb[d][:],
                    mybir.AluOpType.max,
                    mybir.AluOpType.add,
                )
                fp.append(ft)
                st = p_sq.tile([128, RB], BF16, name="sq", tag="sq")
                nc.scalar.square(st[:], ft[:])
                sq.append(st)
            # contiguous PE accumulation group (interleaving a group with
            # other matmul groups breaks PSUM accumulation on HW)
            for d in range(KD):
                nc.tensor.matmul(
                    ps[:],
                    ones_col[:],
                    sq[d][:],
                    start=(d == 0),
                    stop=(d == KD - 1),
                )
            S[b]["fp"] = fp
            S[b]["ps"] = ps
            if dbg and b == 0:
                dbg_dump("dbg_fp00", fp[0][:])
                dbg_dump("dbg_sq00", sq[0][:])

        def emit_ps_norm(b):
            ps = S[b]["ps"]
            iv = p_nrm.tile([1, RB], F32R, name="iv", tag="iv")
            nc.scalar.activation(
                iv[:],
                ps[:],
                mybir.ActivationFunctionType.Abs_reciprocal_sqrt,
                scale=sc_val,
            )
            S[b]["iv"] = iv
            if dbg and b == 0:
                dbg_dump("dbg_iv0", _f32(iv[:]))

        def emit_pb(b):
            iv = S[b]["iv"]
            pb = ps_l.tile([128, RB], F32, name="pl", tag="pl")
            nc.tensor.matmul(pb[:], ones_row[:], iv[:], start=True, stop=True)
            pbs = p_pbs.tile([128, RB], F32, name="pbs", tag="pbs")
            nc.scalar.copy(pbs[:], pb[:])
            S[b]["pbs"] = pbs
            if dbg and b == 0:
                dbg_dump("dbg_pbs0", pbs[:])

        def emit_mm3(b):
            c0 = b * RB
            fp = S[b]["fp"]
            pbs = S[b]["pbs"]
            for t_i in range(TTI):
                pl = ps_l.tile([128, RB], F32, name="pl", tag="pl")
                for k in range(KD):
                    nc.tensor.matmul(
                        pl[:],
                        ttt[k][:, t_i * 128 : (t_i + 1) * 128],
                        fp[k][:],
                        start=(k == 0),
                        stop=(k == KD - 1),
                    )
                ob = p_ob.tile([128, RB], F32, name="ob", tag="ob")
                nc.vector.tensor_mul(ob[:], pl[:], pbs[:])
                nc.sync.dma_start(
                    ot[t_i * 128 : (t_i + 1) * 128, c0 : c0 + RB], ob[:]
                )
            S[b].clear()

        # ---- emission schedule (software pipelined) ---------------------
        if dbg:
            def emit_dbg_consts():
                dbg_dump("dbg_w100", w1t[0][0][:])
                dbg_dump("dbg_tt0", ttt[0][:])
        emit_const_loads()
        emit_w_loads(0)
        emit_loads(0)
        emit_w_loads(1)
        emit_loads(1)
        emit_tt_loads()
        emit_w_loads(2)
        emit_w_loads(3)
        emit_w_loads(NB)  # tail expert B
        if dbg:
            emit_dbg_consts()
        emit_mm1_g(0)
        emit_mm2(0)
        emit_ps_norm(0)
        for b in range(NB):
            if b + 2 < NB:
                emit_loads(b + 2)
            if b + 1 < NB:
                emit_mm1_g(b + 1)
            emit_pb(b)
            if b + 1 < NB:
                emit_mm2(b + 1)
            emit_mm3(b)
            if b + 1 < NB:
                emit_ps_norm(b + 1)

    nc.compile()
    return nc


_NC_CACHE = {}


def _get_program(sc_val):
    if sc_val not in _NC_CACHE:
        _NC_CACHE[sc_val] = build_program(sc_val)
    return _NC_CACHE[sc_val]


def _plan_routing(labels):
    """Partition rows into 8 cores x (3 pure blocks + 1 <=2-domain tail).

    Returns (blocks, tails) where blocks is a list of NC*NPURE
    (expert, rows[RB]) and tails a list of NC (eA, rowsA, eB, rowsB).
    """
    labels = np.asarray(labels).astype(np.int64)
    counts = np.bincount(labels, minlength=ND)
    assert counts.size == ND, "unexpected domain label outside [0, ND)"
    order = np.argsort(labels, kind="stable")
    starts = np.concatenate([[0], np.cumsum(counts)])
    idx = [order[starts[n] : starts[n + 1]] for n in range(ND)]

    tail_total = NC * RB
    big = int(np.argmax(counts))
    t = [int(counts[n] % RB) if n != big else 0 for n in range(ND)]
    t[big] = tail_total - sum(t)
    assert 0 <= t[big] <= counts[big], "domain distribution too skewed to pack"
    assert (counts[big] - t[big]) % RB == 0

    blocks = []
    for n in range(ND):
        npure = (int(counts[n]) - t[n]) // RB
        for i in range(npure):
            blocks.append((n, idx[n][i * RB : (i + 1) * RB]))
    assert len(blocks) == NC * NPURE

    tails = []
    bigpool = idx[big][int(counts[big]) - t[big] :]
    pos = 0
    for n in range(ND):
        if n == big or t[n] == 0:
            continue
        need = RB - t[n]
        tails.append(
            (n, idx[n][int(counts[n]) - t[n] :], big, bigpool[pos : pos + need])
        )
        pos += need
    while len(tails) < NC:
        tails.append((big, bigpool[pos : pos + RB], big, bigpool[pos:pos]))
        pos += RB
    assert pos == t[big] and len(tails) == NC
    return blocks, tails


def _prepare(image_features, domain_labels, W1, W2, text_features, logit_scale):
    image_features = np.asarray(image_features, dtype=np.float32)
    W1 = np.asarray(W1, dtype=np.float32)
    W2 = np.asarray(W2, dtype=np.float32)
    text_features = np.asarray(text_features, dtype=np.float32)
    ls = float(np.asarray(logit_scale))

    blocks, tails = _plan_routing(domain_labels)

    w1b = W1.astype(NP_BF16)
    w2b = (0.25 * W2).astype(NP_BF16)  # fold 0.2/0.8
    tt_full = np.zeros((D, NTP), dtype=NP_BF16)
    tt_full[:, :NT] = text_features.T.astype(NP_BF16)
    sc_val = float(np.float32(np.exp(-2.0 * ls)))

    in_maps = []
    perm = np.empty(B, dtype=np.int64)
    for c in range(NC):
        cblk = blocks[c * NPURE : (c + 1) * NPURE]
        eA, rowsA, eB, rowsB = tails[c]
        nA = len(rowsA)
        rows = np.concatenate([b[1] for b in cblk] + [rowsA, rowsB])
        assert rows.size == BPC
        perm[c * BPC : (c + 1) * BPC] = rows
        experts = [b[0] for b in cblk] + [eA]
        mkc = np.zeros((2, RB), dtype=np.float32)
        mkc[0, :nA] = 1.0
        mkc[1, nA:] = 1.0
        in_maps.append(
            {
                "xt": np.ascontiguousarray(
                    image_features[rows].T.astype(NP_BF16)
                ),
                "w1p": np.ascontiguousarray(w1b[experts]),
                "w1s": w1b[eB],
                "w2p": np.ascontiguousarray(w2b[experts]),
                "w2s": w2b[eB],
                "mk": mkc,
                "tt": tt_full,
            }
        )
    return in_maps, perm, sc_val


def make_in_maps(image_features, domain_labels, W1, W2, text_features, logit_scale):
    in_maps, _, _ = _prepare(
        image_features, domain_labels, W1, W2, text_features, logit_scale
    )
    return in_maps


def kernel(image_features, domain_labels, W1, W2, text_features, logit_scale, **kw):
    in_maps, perm, sc_val = _prepare(
        image_features, domain_labels, W1, W2, text_features, logit_scale
    )
    nc = _get_program(sc_val)
    res = run_bass_kernel_spmd(nc, in_maps, list(range(NC)))

    out = np.empty((B, NT), dtype=np.float32)
    for c in range(NC):
        out[perm[c * BPC : (c + 1) * BPC], :] = res.results[c]["ot"][:NT, :].T
    return out
